# revision 18
# baseline (speedup 1.0000x reference)
"""Trainium2 Bass kernel for a dense transformer block (LN -> causal MHA -> LN -> MLP).

Full shapes: x [2, 2048, 1024], 16 heads (dk=64), MLP hidden 4096, fp32.

Sharding (8 cores): data-parallel over batch (2 groups of 4 cores), tensor-parallel
within each group: 4 heads per core for attention (Megatron column-split QKV,
row-split W_o partial + ReduceScatter over token shards), then the post-attention
residual/LN2/MLP runs token-parallel (512 tokens per core, full MLP weights
streamed from HBM).  The ReduceScatter is the only collective; final output
shards are gathered on the host.

Compute runs feature-major (contraction dim on SBUF partitions) with fp32r
matmuls.  Softmax uses no max-subtraction (scores are tiny: |s| < ~3), causal
masking multiplies exp() by a 0/1 mask, and the softmax denominator comes from
an appended ones-column on V (row 64 of the AV psum accumulator).
"""

import numpy as np

import concourse.bass as bass
import concourse.mybir as mybir
import concourse.tile as tile
from concourse import bacc
from concourse import bass_utils
from concourse.masks import make_identity

B, S, H, NH, DK, FF = 2, 2048, 1024, 16, 64, 4096
TP, DP, NCORES = 4, 2, 8
TS = S // TP  # 512 tokens per core in the token-parallel phase
FQ = (NH // TP) * DK  # 256 q/k/v features per core (4 heads)
EPS = 1e-5
GROUPS = [[0, 1, 2, 3], [4, 5, 6, 7]]

F32 = mybir.dt.float32
F32R = mybir.dt.float32r
AF = mybir.ActivationFunctionType
OP = mybir.AluOpType
AX = mybir.AxisListType


def build(nc):
    d = lambda name, shape: nc.dram_tensor(name, shape, F32, kind="ExternalInput").ap()
    dr = lambda name, shape: nc.dram_tensor(name, shape, F32R, kind="ExternalInput").ap()
    x = d("x", [S, H])
    xs = d("xs", [TS, H])
    wq = dr("wq", [H, FQ])
    wk = dr("wk", [H, FQ])
    wv = dr("wv", [H, FQ])
    bq = d("bq", [2, 128])
    bk = d("bk", [2, 128])
    bv = d("bv", [1, FQ])
    ln1wb = d("ln1wb", [2, H])
    ln2wb = d("ln2wb", [2, H])
    cmask = dr("cmask", [128, 4 * TS])
    wo = dr("wo", [FQ, H])
    bo = d("bo", [1, H])
    wfc = dr("wfc", [H, FF])
    bfc = d("bfc", [FF // 128, 128])
    wproj = dr("wproj", [FF, H])
    bproj = d("bproj", [1, H])
    out = nc.dram_tensor("out", [TS, H], F32, kind="ExternalOutput").ap()

    opart = nc.dram_tensor("opart", [S, H], F32, kind="Internal").ap()
    rdram = nc.dram_tensor("rdram", [16, TS], F32, kind="Internal").ap()
    oshard = nc.dram_tensor("oshard", [TS, H], F32, kind="Internal").ap()

    with tile.TileContext(nc) as tc:
        _build(tc, x, xs, wq, wk, wv, bq, bk, bv, ln1wb, ln2wb, cmask, wo, bo,
               wfc, bfc, wproj, bproj, out, opart, oshard, rdram)
    return nc


def _layernorm_tile(nc, pools, xt, w_b, b_b, eps_sb, width=H):
    """xt: [128, width] SBUF -> returns ln tile [128, width]."""
    stats, scratch, lnp = pools
    s1 = stats.tile([128, 1], F32, tag="s1")
    nc.vector.reduce_sum(s1[:], xt[:], axis=AX.X)
    sq = scratch.tile([128, width], F32, tag="sq")
    s2 = stats.tile([128, 1], F32, tag="s2")
    nc.scalar.activation(sq[:], xt[:], AF.Square, accum_out=s2[:])
    negmu = stats.tile([128, 1], F32, tag="negmu")
    nc.scalar.mul(negmu[:], s1[:], -1.0 / width)
    mu2 = stats.tile([128, 1], F32, tag="mu2")
    nc.scalar.activation(mu2[:], s1[:], AF.Square, scale=1.0 / width)
    nmu2 = stats.tile([128, 1], F32, tag="nmu2")
    # nmu2 = EPS - mu2  (var = s2/width - mu^2; +EPS folded in)
    nc.scalar.activation(nmu2[:], mu2[:], AF.Identity, scale=-1.0, bias=eps_sb[:])
    std = stats.tile([128, 1], F32, tag="std")
    nc.scalar.activation(std[:], s2[:], AF.Sqrt, scale=1.0 / width, bias=nmu2[:])
    rinv = stats.tile([128, 1], F32, tag="rinv")
    nc.vector.reciprocal(rinv[:], std[:])
    t = scratch.tile([128, width], F32, tag="lt")
    nc.vector.scalar_tensor_tensor(t[:], xt[:], negmu[:], w_b[:], OP.add, OP.mult)
    lnt = lnp.tile([128, width], F32, tag="ln")
    nc.vector.scalar_tensor_tensor(lnt[:], t[:], rinv[:], b_b[:], OP.mult, OP.add)
    return lnt


def _build(tc, x, xs, wq, wk, wv, bq, bk, bv, ln1wb, ln2wb, cmask, wo, bo,
           wfc, bfc, wproj, bproj, out, opart, oshard, rdram):
    nc = tc.nc
    ctx_consts = tc.tile_pool(name="consts", bufs=1)
    with ctx_consts as consts:
        ident = consts.tile([128, 128], F32, tag="ident")
        make_identity(nc, ident[:])
        eps_sb = consts.tile([128, 1], F32, tag="eps")
        nc.gpsimd.memset(eps_sb[:], EPS)

        with tc.tile_pool(name="qkvout", bufs=1) as qkvout:
            QT = qkvout.tile([128, 2, S], F32R, tag="QT")
            KT = qkvout.tile([128, 2, S], F32R, tag="KT")
            VA = qkvout.tile([128, 16, 4, 65], F32R, tag="VA")
            AOT = qkvout.tile([128, 2, S], F32R, tag="AOT")
            nc.gpsimd.memset(VA[:, :, :, 64:65].bitcast(F32), 1.0)

            # ---------------- Phase A: LN1 + transpose + QKV ----------------
            with (
                tc.tile_pool(name="xin", bufs=2) as xin,
                tc.tile_pool(name="lnb", bufs=3) as lnp,
                tc.tile_pool(name="lnT", bufs=2) as lnTp,
                tc.tile_pool(name="wqkv", bufs=1) as wqkv,
                tc.tile_pool(name="stats", bufs=6) as stats,
                tc.tile_pool(name="scratch", bufs=2) as scratch,
                tc.tile_pool(name="constsA", bufs=1) as cA,
                tc.tile_pool(name="psA", bufs=3, space="PSUM") as psA,
                tc.tile_pool(name="psT", bufs=2, space="PSUM") as psT,
                tc.tile_pool(name="psV", bufs=2, space="PSUM") as psV,
            ):
                w1b = cA.tile([128, H], F32, tag="w1b")
                nc.sync.dma_start(w1b[:], ln1wb[0, :].partition_broadcast(128))
                b1b = cA.tile([128, H], F32, tag="b1b")
                nc.sync.dma_start(b1b[:], ln1wb[1, :].partition_broadcast(128))
                bq_sb = cA.tile([128, 2], F32, tag="bq")
                nc.sync.dma_start(bq_sb[:], bq.rearrange("c p -> p c"))
                bk_sb = cA.tile([128, 2], F32, tag="bk")
                nc.sync.dma_start(bk_sb[:], bk.rearrange("c p -> p c"))
                bv_sb = cA.tile([128, FQ], F32, tag="bv")
                nc.sync.dma_start(bv_sb[:], bv[0, :].partition_broadcast(128))

                wq_sb = wqkv.tile([128, 8, FQ], F32R, tag="wq")
                nc.sync.dma_start(wq_sb[:], wq.rearrange("(h p) f -> p h f", p=128))
                wk_sb = wqkv.tile([128, 8, FQ], F32R, tag="wk")
                nc.sync.dma_start(wk_sb[:], wk.rearrange("(h p) f -> p h f", p=128))
                wv_sb = wqkv.tile([128, 8, FQ], F32R, tag="wv")
                nc.sync.dma_start(wv_sb[:], wv.rearrange("(h p) f -> p h f", p=128))

                lnpools = (stats, scratch, lnp)

                for tcn in range(4):
                    lnT = lnTp.tile([128, 8, TS], F32R, tag="lnT",
                                    name=f"lnT{tcn}")
                    for u in range(4):
                        t = 4 * tcn + u
                        xt = xin.tile([128, H], F32, tag="x")
                        nc.sync.dma_start(xt[:], x[128 * t : 128 * (t + 1), :])
                        lnt = _layernorm_tile(nc, lnpools, xt, w1b, b1b, eps_sb)
                        for f in range(8):
                            pt = psT.tile([128, 128], F32, tag="pt")
                            nc.tensor.transpose(
                                pt[:], lnt[:, 128 * f : 128 * (f + 1)], ident[:]
                            )
                            nc.vector.tensor_copy(
                                lnT[:, f, 128 * u : 128 * (u + 1)], pt[:]
                            )
                    # Q/K feature-major chunks for this 512-token column
                    tsl = slice(TS * tcn, TS * (tcn + 1))
                    for (wt, dst, bias) in ((wq_sb, QT, bq_sb), (wk_sb, KT, bk_sb)):
                        for c in range(2):
                            pq = psA.tile([128, TS], F32, tag="pq")
                            for ht in range(8):
                                nc.tensor.matmul(
                                    pq[:],
                                    wt[:, ht, 128 * c : 128 * (c + 1)],
                                    lnT[:, ht, :],
                                    start=(ht == 0),
                                    stop=(ht == 7),
                                )
                            nc.scalar.activation(
                                dst[:, c, tsl], pq[:], AF.Identity,
                                bias=bias[:, c : c + 1],
                            )
                    # V token-major for the 4 token tiles of this column
                    for u in range(4):
                        t = 4 * tcn + u
                        pv = psV.tile([128, FQ], F32, tag="pvq")
                        for ht in range(8):
                            nc.tensor.matmul(
                                pv[:],
                                lnT[:, ht, 128 * u : 128 * (u + 1)],
                                wv_sb[:, ht, :],
                                start=(ht == 0),
                                stop=(ht == 7),
                            )
                        nc.vector.tensor_add(
                            VA[:, t, :, 0:64],
                            pv[:].rearrange("p (h f) -> p h f", h=4),
                            bv_sb[:].rearrange("p (h f) -> p h f", h=4),
                        )

        # ---------------- Phase B: attention + partial O-proj ----------------
            with (
                tc.tile_pool(name="attnB", bufs=1) as aB,
                tc.tile_pool(name="epool", bufs=6) as epool,
                tc.tile_pool(name="rpool", bufs=4) as rpool,
                tc.tile_pool(name="psS", bufs=4, space="PSUM") as psS,
                tc.tile_pool(name="psAV", bufs=2, space="PSUM") as psAV,
                tc.tile_pool(name="psO", bufs=2, space="PSUM") as psO,
            ):
                mask_sb = aB.tile([128, 4, TS], F32R, tag="mask")
                nc.sync.dma_start(
                    mask_sb[:], cmask.rearrange("p (d q) -> p d q", d=4)
                )
                wo_sb = aB.tile([128, 2, H], F32R, tag="wo")
                nc.sync.dma_start(wo_sb[:], wo.rearrange("(c p) f -> p c f", p=128))

                for qc in range(4):
                    qsl = slice(TS * qc, TS * (qc + 1))
                    nkb = 4 * qc + 4
                    for hp in range(2):
                        pvs = [
                            psAV.tile([65, TS], F32, tag="pav",
                                      name=f"pav{qc}_{hp}_{i}")
                            for i in range(2)
                        ]
                        for kb in range(nkb):
                            es = []
                            for hh in range(2):
                                base = 64 * hh
                                sp = psS.tile([128, TS], F32, tag="sp")
                                nc.tensor.matmul(
                                    sp[:],
                                    KT[base : base + 64, hp,
                                          128 * kb : 128 * (kb + 1)],
                                    QT[base : base + 64, hp, qsl],
                                    start=True,
                                    stop=True,
                                    tile_position=(base, 0),
                                )
                                e = epool.tile([128, TS], F32R, tag="e")
                                nc.scalar.activation(e[:], sp[:], AF.Exp, scale=0.125)
                                dd = kb - 4 * qc
                                if dd >= 0:
                                    nc.vector.tensor_mul(
                                        e[:], e[:], mask_sb[:, dd, :]
                                    )
                                es.append(e)
                            for hh in range(2):
                                h = 2 * hp + hh
                                nc.tensor.matmul(
                                    pvs[hh][:],
                                    VA[:, kb, h, :],
                                    es[hh][:],
                                    start=(kb == 0),
                                    stop=(kb == nkb - 1),
                                    skip_group_check=True,
                                )
                        for hh in range(2):
                            base = 64 * hh
                            rd = rpool.tile([1, TS], F32, tag="rd")
                            nc.vector.reciprocal(rd[:], pvs[hh][64:65, :])
                            ri = 4 * qc + 2 * hp + hh
                            nc.sync.dma_start(rdram[ri : ri + 1, :], rd[:])
                            rdb = rpool.tile([64, TS], F32, tag="rdb")
                            nc.sync.dma_start(
                                rdb[:], rdram[ri, :].partition_broadcast(64)
                            )
                            nc.vector.tensor_mul(
                                AOT[base : base + 64, hp, qsl],
                                pvs[hh][0:64, :],
                                rdb[:],
                            )
                    # partial O-projection for this token column (all 4 heads)
                    for u in range(4):
                        t = 4 * qc + u
                        for oc in range(2):
                            po = psO.tile([128, TS], F32, tag="po")
                            for c in range(2):
                                nc.tensor.matmul(
                                    po[:],
                                    AOT[:, c, 128 * t : 128 * (t + 1)],
                                    wo_sb[:, c, TS * oc : TS * (oc + 1)],
                                    start=(c == 0),
                                    stop=(c == 1),
                                )
                            ost = epool.tile([128, TS], F32, tag="ost")
                            nc.vector.tensor_copy(ost[:], po[:])
                            nc.sync.dma_start(
                                opart[128 * t : 128 * (t + 1),
                                      TS * oc : TS * (oc + 1)],
                                ost[:],
                            )

        # ---------------- ReduceScatter ----------------
        nc.gpsimd.collective_compute(
            "ReduceScatter",
            OP.add,
            replica_groups=GROUPS,
            ins=[opart[:]],
            outs=[oshard[:]],
        )

        # ---------------- Phase C: residual + LN2 + MLP (token-parallel) ----
        with (
            tc.tile_pool(name="x2p", bufs=1) as x2p,
            tc.tile_pool(name="cin", bufs=2) as cin,
            tc.tile_pool(name="lnb2", bufs=2) as lnp2,
            tc.tile_pool(name="ln2T", bufs=1) as ln2Tp,
            tc.tile_pool(name="h1p", bufs=1) as h1p,
            tc.tile_pool(name="wstream", bufs=2) as wstream,
            tc.tile_pool(name="stats2", bufs=6) as stats2,
            tc.tile_pool(name="scratch2", bufs=1) as scratch2,
            tc.tile_pool(name="outp", bufs=2) as outp,
            tc.tile_pool(name="constsC", bufs=1) as cC,
        ):
            w2b = cC.tile([128, H], F32, tag="w2b")
            nc.sync.dma_start(w2b[:], ln2wb[0, :].partition_broadcast(128))
            b2b = cC.tile([128, H], F32, tag="b2b")
            nc.sync.dma_start(b2b[:], ln2wb[1, :].partition_broadcast(128))
            bo_b = cC.tile([128, H], F32, tag="bo_b")
            nc.sync.dma_start(bo_b[:], bo[0, :].partition_broadcast(128))
            bproj_b = cC.tile([128, H], F32, tag="bproj_b")
            nc.sync.dma_start(bproj_b[:], bproj[0, :].partition_broadcast(128))
            bfc_sb = cC.tile([128, FF // 128], F32, tag="bfc")
            nc.sync.dma_start(bfc_sb[:], bfc.rearrange("c p -> p c"))

            x2 = x2p.tile([128, 4, H], F32, tag="x2")
            ln2T = ln2Tp.tile([128, 8, TS], F32R, tag="ln2T")
            h1T = h1p.tile([128, 32, TS], F32R, tag="h1T")
            lnpools2 = (stats2, scratch2, lnp2)

            with (
                tc.tile_pool(name="psC", bufs=3, space="PSUM") as psC,
                tc.tile_pool(name="psT2", bufs=2, space="PSUM") as psT2,
            ):
                for u in range(4):
                    ot = cin.tile([128, H], F32, tag="ot")
                    nc.sync.dma_start(ot[:], oshard[128 * u : 128 * (u + 1), :])
                    xst = cin.tile([128, H], F32, tag="xst")
                    nc.sync.dma_start(xst[:], xs[128 * u : 128 * (u + 1), :])
                    t1 = scratch2.tile([128, H], F32, tag="t1")
                    nc.vector.tensor_add(t1[:], ot[:], xst[:])
                    nc.vector.tensor_add(x2[:, u, :], t1[:], bo_b[:])
                    lnt = _layernorm_tile(nc, lnpools2, x2[:, u, :], w2b, b2b, eps_sb)
                    for f in range(8):
                        pt = psT2.tile([128, 128], F32, tag="pt2")
                        nc.tensor.transpose(
                            pt[:], lnt[:, 128 * f : 128 * (f + 1)], ident[:]
                        )
                        nc.vector.tensor_copy(
                            ln2T[:, f, 128 * u : 128 * (u + 1)], pt[:]
                        )

                # FC + gelu, feature-major
                for g in range(8):
                    wt = wstream.tile([128, 8, TS], F32R, tag="wst")
                    nc.sync.dma_start(
                        wt[:],
                        wfc.rearrange("(h p) f -> p h f", p=128)[
                            :, :, TS * g : TS * (g + 1)
                        ],
                    )
                    for c4 in range(4):
                        fc = 4 * g + c4
                        pf = psC.tile([128, TS], F32, tag="pf")
                        for ht in range(8):
                            nc.tensor.matmul(
                                pf[:],
                                wt[:, ht, 128 * c4 : 128 * (c4 + 1)],
                                ln2T[:, ht, :],
                                start=(ht == 0),
                                stop=(ht == 7),
                            )
                        nc.scalar.activation(
                            h1T[:, fc, :], pf[:], AF.Gelu, bias=bfc_sb[:, fc : fc + 1]
                        )

            # proj back to H, token-major, 8 accumulators across streamed weights
            with tc.tile_pool(name="psP", bufs=8, space="PSUM") as psP:
                pps = [psP.tile([128, TS], F32, tag="pp", name=f"pp{i}")
                       for i in range(8)]
                for g in range(8):
                    wt = wstream.tile([128, 4, H], F32R, tag="wst")
                    nc.sync.dma_start(
                        wt[:],
                        wproj.rearrange("(c p) f -> p c f", p=128)[
                            :, 4 * g : 4 * (g + 1), :
                        ],
                    )
                    for u in range(4):
                        for oc in range(2):
                            for f4 in range(4):
                                fc = 4 * g + f4
                                nc.tensor.matmul(
                                    pps[2 * u + oc][:],
                                    h1T[:, fc, 128 * u : 128 * (u + 1)],
                                    wt[:, f4, TS * oc : TS * (oc + 1)],
                                    start=(g == 0 and f4 == 0),
                                    stop=(g == 7 and f4 == 3),
                                    skip_group_check=True,
                                )
                for u in range(4):
                    for oc in range(2):
                        osl = slice(TS * oc, TS * (oc + 1))
                        t1 = scratch2.tile([128, H], F32, tag="t1")
                        nc.vector.tensor_add(
                            t1[:, 0:TS], pps[2 * u + oc][:], x2[:, u, osl]
                        )
                        ro = outp.tile([128, TS], F32, tag="ro")
                        nc.vector.tensor_add(ro[:], t1[:, 0:TS], bproj_b[:, osl])
                        nc.sync.dma_start(out[128 * u : 128 * (u + 1), osl], ro[:])


_CACHE = {}


def _get_compiled():
    if "nc" not in _CACHE:
        nc = bacc.Bacc("TRN2", target_bir_lowering=False, debug=False,
                       num_devices=NCORES)
        build(nc)
        nc.compile()
        _CACHE["nc"] = nc
    return _CACHE["nc"]


def _make_cmask():
    i = np.arange(128)[:, None]
    j = np.arange(TS)[None, :]
    m = np.zeros((128, 4, TS), np.float32)
    for dd in range(4):
        m[:, dd, :] = (128 * dd + i <= j).astype(np.float32)
    return np.ascontiguousarray(m.reshape(128, 4 * TS))


def kernel(x, ln1_w, ln1_b, W_qkv, b_qkv, W_o, b_o, ln2_w, ln2_b, W_fc, b_fc,
           W_proj, b_proj):
    x = np.ascontiguousarray(np.asarray(x, np.float32))
    W_qkv = np.asarray(W_qkv, np.float32)
    b_qkv = np.asarray(b_qkv, np.float32)
    nc = _get_compiled()
    cm = _make_cmask()
    ln1wb = np.stack([np.asarray(ln1_w, np.float32), np.asarray(ln1_b, np.float32)])
    ln2wb = np.stack([np.asarray(ln2_w, np.float32), np.asarray(ln2_b, np.float32)])
    shared = {
        "ln1wb": ln1wb,
        "ln2wb": ln2wb,
        "cmask": cm,
        "bo": np.asarray(b_o, np.float32).reshape(1, H),
        "wfc": np.ascontiguousarray(np.asarray(W_fc, np.float32)),
        "bfc": np.ascontiguousarray(np.asarray(b_fc, np.float32).reshape(FF // 128, 128)),
        "wproj": np.ascontiguousarray(np.asarray(W_proj, np.float32)),
        "bproj": np.asarray(b_proj, np.float32).reshape(1, H),
    }
    in_maps = []
    for c in range(NCORES):
        b, r = c // TP, c % TP
        fsl = slice(FQ * r, FQ * (r + 1))
        m = dict(shared)
        m["x"] = x[b]
        m["xs"] = np.ascontiguousarray(x[b][TS * r : TS * (r + 1)])
        m["wq"] = np.ascontiguousarray(W_qkv[:, fsl])
        m["wk"] = np.ascontiguousarray(W_qkv[:, H:][:, fsl])
        m["wv"] = np.ascontiguousarray(W_qkv[:, 2 * H :][:, fsl])
        m["bq"] = np.ascontiguousarray(b_qkv[fsl].reshape(2, 128))
        m["bk"] = np.ascontiguousarray(b_qkv[H:][fsl].reshape(2, 128))
        m["bv"] = np.ascontiguousarray(b_qkv[2 * H :][fsl].reshape(1, FQ))
        m["wo"] = np.ascontiguousarray(np.asarray(W_o, np.float32)[fsl, :])
        in_maps.append(m)

    res = bass_utils.run_bass_kernel_spmd(
        nc, in_maps, core_ids=list(range(NCORES)), trace=False
    )
    out = np.empty((B, S, H), np.float32)
    for c in range(NCORES):
        b, r = c // TP, c % TP
        out[b, TS * r : TS * (r + 1), :] = res.results[c]["out"]
    return out


# revision 20
# speedup vs baseline: 932.4326x; 932.4326x over previous
"""Trainium2 Bass kernel for a dense transformer block (LN -> causal MHA -> LN -> MLP).

Full shapes: x [2, 2048, 1024], 16 heads (dk=64), MLP hidden 4096, fp32.

Sharding (8 cores): data-parallel over batch (2 groups of 4 cores), tensor-parallel
within each group: 4 heads per core for attention (Megatron column-split QKV,
row-split W_o partial + ReduceScatter over token shards), then the post-attention
residual/LN2/MLP runs token-parallel (512 tokens per core, full MLP weights
streamed from HBM).  The ReduceScatter is the only collective; final output
shards are gathered on the host.

Compute runs feature-major (contraction dim on SBUF partitions) with fp32r
matmuls.  Softmax uses no max-subtraction (scores are tiny: |s| < ~3), causal
masking multiplies exp() by a 0/1 mask, and the softmax denominator comes from
an appended ones-column on V (row 64 of the AV psum accumulator).
"""

import numpy as np

import concourse.bass as bass
import concourse.mybir as mybir
import concourse.tile as tile
from concourse import bacc
from concourse import bass_utils
from concourse.masks import make_identity

B, S, H, NH, DK, FF = 2, 2048, 1024, 16, 64, 4096
TP, DP, NCORES = 4, 2, 8
TS = S // TP  # 512 tokens per core in the token-parallel phase
FQ = (NH // TP) * DK  # 256 q/k/v features per core (4 heads)
EPS = 1e-5
GROUPS = [[0, 1, 2, 3], [4, 5, 6, 7]]

F32 = mybir.dt.float32
F32R = mybir.dt.float32r
AF = mybir.ActivationFunctionType
OP = mybir.AluOpType
AX = mybir.AxisListType


def build(nc):
    d = lambda name, shape: nc.dram_tensor(name, shape, F32, kind="ExternalInput").ap()
    dr = lambda name, shape: nc.dram_tensor(name, shape, F32R, kind="ExternalInput").ap()
    x = d("x", [S, H])
    xs = d("xs", [TS, H])
    wq = dr("wq", [H, FQ])
    wk = dr("wk", [H, FQ])
    wv = dr("wv", [H, FQ])
    bq = d("bq", [2, 128])
    bk = d("bk", [2, 128])
    bv = d("bv", [1, FQ])
    ln1wb = d("ln1wb", [2, H])
    ln2wb = d("ln2wb", [2, H])
    cmask = dr("cmask", [128, 4 * TS])
    wo = dr("wo", [FQ, H])
    bo = d("bo", [1, H])
    wfc = dr("wfc", [H, FF])
    bfc = d("bfc", [FF // 128, 128])
    wproj = dr("wproj", [FF, H])
    bproj = d("bproj", [1, H])
    out = nc.dram_tensor("out", [TS, H], F32, kind="ExternalOutput").ap()

    opart = nc.dram_tensor("opart", [S, H], F32, kind="Internal").ap()
    rdram = nc.dram_tensor("rdram", [16, TS], F32, kind="Internal").ap()
    oshard = nc.dram_tensor("oshard", [TS, H], F32, kind="Internal").ap()

    with tile.TileContext(nc) as tc:
        _build(tc, x, xs, wq, wk, wv, bq, bk, bv, ln1wb, ln2wb, cmask, wo, bo,
               wfc, bfc, wproj, bproj, out, opart, oshard, rdram)
    return nc


def _layernorm_tile(nc, pools, xt, w_b, b_b, eps_sb, width=H):
    """xt: [128, width] SBUF -> returns ln tile [128, width]."""
    stats, scratch, lnp = pools
    s1 = stats.tile([128, 1], F32, tag="s1")
    nc.vector.reduce_sum(s1[:], xt[:], axis=AX.X)
    sq = scratch.tile([128, width], F32, tag="sq")
    s2 = stats.tile([128, 1], F32, tag="s2")
    nc.scalar.activation(sq[:], xt[:], AF.Square, accum_out=s2[:])
    negmu = stats.tile([128, 1], F32, tag="negmu")
    nc.scalar.mul(negmu[:], s1[:], -1.0 / width)
    mu2 = stats.tile([128, 1], F32, tag="mu2")
    nc.scalar.activation(mu2[:], s1[:], AF.Square, scale=1.0 / width)
    nmu2 = stats.tile([128, 1], F32, tag="nmu2")
    # nmu2 = EPS - mu2  (var = s2/width - mu^2; +EPS folded in)
    nc.scalar.activation(nmu2[:], mu2[:], AF.Identity, scale=-1.0, bias=eps_sb[:])
    std = stats.tile([128, 1], F32, tag="std")
    nc.scalar.activation(std[:], s2[:], AF.Sqrt, scale=1.0 / width, bias=nmu2[:])
    rinv = stats.tile([128, 1], F32, tag="rinv")
    nc.vector.reciprocal(rinv[:], std[:])
    t = scratch.tile([128, width], F32, tag="lt")
    nc.vector.scalar_tensor_tensor(t[:], xt[:], negmu[:], w_b[:], OP.add, OP.mult)
    lnt = lnp.tile([128, width], F32, tag="ln")
    nc.vector.scalar_tensor_tensor(lnt[:], t[:], rinv[:], b_b[:], OP.mult, OP.add)
    return lnt


def _build(tc, x, xs, wq, wk, wv, bq, bk, bv, ln1wb, ln2wb, cmask, wo, bo,
           wfc, bfc, wproj, bproj, out, opart, oshard, rdram):
    nc = tc.nc
    ctx_consts = tc.tile_pool(name="consts", bufs=1)
    with ctx_consts as consts:
        ident = consts.tile([128, 128], F32, tag="ident")
        make_identity(nc, ident[:])
        eps_sb = consts.tile([128, 1], F32, tag="eps")
        nc.gpsimd.memset(eps_sb[:], EPS)

        with tc.tile_pool(name="qkvout", bufs=1) as qkvout:
            QT = qkvout.tile([128, 2, S], F32R, tag="QT")
            KT = qkvout.tile([128, 2, S], F32R, tag="KT")
            VA = qkvout.tile([128, 16, 4, 65], F32R, tag="VA")
            AOT = qkvout.tile([128, 2, S], F32R, tag="AOT")
            nc.gpsimd.memset(VA[:, :, :, 64:65].bitcast(F32), 1.0)

            # ---------------- Phase A: LN1 + transpose + QKV ----------------
            with (
                tc.tile_pool(name="xin", bufs=2) as xin,
                tc.tile_pool(name="lnb", bufs=3) as lnp,
                tc.tile_pool(name="lnT", bufs=2) as lnTp,
                tc.tile_pool(name="wqkv", bufs=1) as wqkv,
                tc.tile_pool(name="stats", bufs=6) as stats,
                tc.tile_pool(name="scratch", bufs=2) as scratch,
                tc.tile_pool(name="constsA", bufs=1) as cA,
                tc.tile_pool(name="psA", bufs=3, space="PSUM") as psA,
                tc.tile_pool(name="psT", bufs=2, space="PSUM") as psT,
                tc.tile_pool(name="psV", bufs=2, space="PSUM") as psV,
            ):
                w1b = cA.tile([128, H], F32, tag="w1b")
                nc.sync.dma_start(w1b[:], ln1wb[0, :].partition_broadcast(128))
                b1b = cA.tile([128, H], F32, tag="b1b")
                nc.sync.dma_start(b1b[:], ln1wb[1, :].partition_broadcast(128))
                bq_sb = cA.tile([128, 2], F32, tag="bq")
                nc.sync.dma_start(bq_sb[:], bq.rearrange("c p -> p c"))
                bk_sb = cA.tile([128, 2], F32, tag="bk")
                nc.sync.dma_start(bk_sb[:], bk.rearrange("c p -> p c"))
                bv_sb = cA.tile([128, FQ], F32, tag="bv")
                nc.sync.dma_start(bv_sb[:], bv[0, :].partition_broadcast(128))

                wq_sb = wqkv.tile([128, 8, FQ], F32R, tag="wq")
                nc.sync.dma_start(wq_sb[:], wq.rearrange("(h p) f -> p h f", p=128))
                wk_sb = wqkv.tile([128, 8, FQ], F32R, tag="wk")
                nc.sync.dma_start(wk_sb[:], wk.rearrange("(h p) f -> p h f", p=128))
                wv_sb = wqkv.tile([128, 8, FQ], F32R, tag="wv")
                nc.sync.dma_start(wv_sb[:], wv.rearrange("(h p) f -> p h f", p=128))

                lnpools = (stats, scratch, lnp)

                for tcn in range(4):
                    lnT = lnTp.tile([128, 8, TS], F32R, tag="lnT",
                                    name=f"lnT{tcn}")
                    for u in range(4):
                        t = 4 * tcn + u
                        xt = xin.tile([128, H], F32, tag="x")
                        nc.sync.dma_start(xt[:], x[128 * t : 128 * (t + 1), :])
                        lnt = _layernorm_tile(nc, lnpools, xt, w1b, b1b, eps_sb)
                        for f in range(8):
                            pt = psT.tile([128, 128], F32, tag="pt")
                            nc.tensor.transpose(
                                pt[:], lnt[:, 128 * f : 128 * (f + 1)], ident[:]
                            )
                            nc.vector.tensor_copy(
                                lnT[:, f, 128 * u : 128 * (u + 1)], pt[:]
                            )
                    # Q/K feature-major chunks for this 512-token column
                    tsl = slice(TS * tcn, TS * (tcn + 1))
                    for (wt, dst, bias) in ((wq_sb, QT, bq_sb), (wk_sb, KT, bk_sb)):
                        for c in range(2):
                            pq = psA.tile([128, TS], F32, tag="pq")
                            for ht in range(8):
                                nc.tensor.matmul(
                                    pq[:],
                                    wt[:, ht, 128 * c : 128 * (c + 1)],
                                    lnT[:, ht, :],
                                    start=(ht == 0),
                                    stop=(ht == 7),
                                )
                            nc.scalar.activation(
                                dst[:, c, tsl], pq[:], AF.Identity,
                                bias=bias[:, c : c + 1],
                            )
                    # V token-major for the 4 token tiles of this column
                    for u in range(4):
                        t = 4 * tcn + u
                        pv = psV.tile([128, FQ], F32, tag="pvq")
                        for ht in range(8):
                            nc.tensor.matmul(
                                pv[:],
                                lnT[:, ht, 128 * u : 128 * (u + 1)],
                                wv_sb[:, ht, :],
                                start=(ht == 0),
                                stop=(ht == 7),
                            )
                        nc.vector.tensor_add(
                            VA[:, t, :, 0:64],
                            pv[:].rearrange("p (h f) -> p h f", h=4),
                            bv_sb[:].rearrange("p (h f) -> p h f", h=4),
                        )

        # ---------------- Phase B: attention + partial O-proj ----------------
            with (
                tc.tile_pool(name="attnB", bufs=1) as aB,
                tc.tile_pool(name="epool", bufs=6) as epool,
                tc.tile_pool(name="rpool", bufs=4) as rpool,
                tc.tile_pool(name="psS", bufs=4, space="PSUM") as psS,
                tc.tile_pool(name="psAV", bufs=2, space="PSUM") as psAV,
                tc.tile_pool(name="psO", bufs=2, space="PSUM") as psO,
            ):
                mask_sb = aB.tile([128, 4, TS], F32R, tag="mask")
                nc.sync.dma_start(
                    mask_sb[:], cmask.rearrange("p (d q) -> p d q", d=4)
                )
                wo_sb = aB.tile([128, 2, H], F32R, tag="wo")
                nc.sync.dma_start(wo_sb[:], wo.rearrange("(c p) f -> p c f", p=128))

                for qc in range(4):
                    qsl = slice(TS * qc, TS * (qc + 1))
                    nkb = 4 * qc + 4
                    for hp in range(2):
                        pvs = [
                            psAV.tile([65, TS], F32, tag="pav",
                                      name=f"pav{qc}_{hp}_{i}")
                            for i in range(2)
                        ]
                        for kb in range(nkb):
                            es = []
                            for hh in range(2):
                                base = 64 * hh
                                sp = psS.tile([128, TS], F32, tag="sp")
                                nc.tensor.matmul(
                                    sp[:],
                                    KT[base : base + 64, hp,
                                          128 * kb : 128 * (kb + 1)],
                                    QT[base : base + 64, hp, qsl],
                                    start=True,
                                    stop=True,
                                    tile_position=(base, 0),
                                )
                                e = epool.tile([128, TS], F32R, tag="e")
                                nc.scalar.activation(e[:], sp[:], AF.Exp, scale=0.125)
                                dd = kb - 4 * qc
                                if dd >= 0:
                                    nc.vector.tensor_mul(
                                        e[:], e[:], mask_sb[:, dd, :]
                                    )
                                es.append(e)
                            for hh in range(2):
                                h = 2 * hp + hh
                                nc.tensor.matmul(
                                    pvs[hh][:],
                                    VA[:, kb, h, :],
                                    es[hh][:],
                                    start=(kb == 0),
                                    stop=(kb == nkb - 1),
                                    skip_group_check=True,
                                )
                        for hh in range(2):
                            base = 64 * hh
                            rd = rpool.tile([1, TS], F32, tag="rd")
                            nc.vector.reciprocal(rd[:], pvs[hh][64:65, :])
                            ri = 4 * qc + 2 * hp + hh
                            nc.sync.dma_start(rdram[ri : ri + 1, :], rd[:])
                            rdb = rpool.tile([64, TS], F32, tag="rdb")
                            nc.sync.dma_start(
                                rdb[:], rdram[ri, :].partition_broadcast(64)
                            )
                            nc.vector.tensor_mul(
                                AOT[base : base + 64, hp, qsl],
                                pvs[hh][0:64, :],
                                rdb[:],
                            )
                    # partial O-projection for this token column (all 4 heads)
                    for u in range(4):
                        t = 4 * qc + u
                        for oc in range(2):
                            po = psO.tile([128, TS], F32, tag="po")
                            for c in range(2):
                                nc.tensor.matmul(
                                    po[:],
                                    AOT[:, c, 128 * t : 128 * (t + 1)],
                                    wo_sb[:, c, TS * oc : TS * (oc + 1)],
                                    start=(c == 0),
                                    stop=(c == 1),
                                )
                            ost = epool.tile([128, TS], F32, tag="ost")
                            nc.vector.tensor_copy(ost[:], po[:])
                            nc.sync.dma_start(
                                opart[128 * t : 128 * (t + 1),
                                      TS * oc : TS * (oc + 1)],
                                ost[:],
                            )

        # ---------------- ReduceScatter ----------------
        nc.gpsimd.collective_compute(
            "ReduceScatter",
            OP.add,
            replica_groups=GROUPS,
            ins=[opart[:]],
            outs=[oshard[:]],
        )

        # ---------------- Phase C: residual + LN2 + MLP (token-parallel) ----
        with (
            tc.tile_pool(name="x2p", bufs=1) as x2p,
            tc.tile_pool(name="cin", bufs=2) as cin,
            tc.tile_pool(name="lnb2", bufs=2) as lnp2,
            tc.tile_pool(name="ln2T", bufs=1) as ln2Tp,
            tc.tile_pool(name="h1p", bufs=1) as h1p,
            tc.tile_pool(name="wstream", bufs=2) as wstream,
            tc.tile_pool(name="stats2", bufs=6) as stats2,
            tc.tile_pool(name="scratch2", bufs=1) as scratch2,
            tc.tile_pool(name="outp", bufs=2) as outp,
            tc.tile_pool(name="constsC", bufs=1) as cC,
        ):
            w2b = cC.tile([128, H], F32, tag="w2b")
            nc.sync.dma_start(w2b[:], ln2wb[0, :].partition_broadcast(128))
            b2b = cC.tile([128, H], F32, tag="b2b")
            nc.sync.dma_start(b2b[:], ln2wb[1, :].partition_broadcast(128))
            bo_b = cC.tile([128, H], F32, tag="bo_b")
            nc.sync.dma_start(bo_b[:], bo[0, :].partition_broadcast(128))
            bproj_b = cC.tile([128, H], F32, tag="bproj_b")
            nc.sync.dma_start(bproj_b[:], bproj[0, :].partition_broadcast(128))
            bfc_sb = cC.tile([128, FF // 128], F32, tag="bfc")
            nc.sync.dma_start(bfc_sb[:], bfc.rearrange("c p -> p c"))

            x2 = x2p.tile([128, 4, H], F32, tag="x2")
            ln2T = ln2Tp.tile([128, 8, TS], F32R, tag="ln2T")
            h1T = h1p.tile([128, 32, TS], F32R, tag="h1T")
            lnpools2 = (stats2, scratch2, lnp2)

            with (
                tc.tile_pool(name="psC", bufs=3, space="PSUM") as psC,
                tc.tile_pool(name="psT2", bufs=2, space="PSUM") as psT2,
            ):
                for u in range(4):
                    ot = cin.tile([128, H], F32, tag="ot")
                    nc.sync.dma_start(ot[:], oshard[128 * u : 128 * (u + 1), :])
                    xst = cin.tile([128, H], F32, tag="xst")
                    nc.sync.dma_start(xst[:], xs[128 * u : 128 * (u + 1), :])
                    t1 = scratch2.tile([128, H], F32, tag="t1")
                    nc.vector.tensor_add(t1[:], ot[:], xst[:])
                    nc.vector.tensor_add(x2[:, u, :], t1[:], bo_b[:])
                    lnt = _layernorm_tile(nc, lnpools2, x2[:, u, :], w2b, b2b, eps_sb)
                    for f in range(8):
                        pt = psT2.tile([128, 128], F32, tag="pt2")
                        nc.tensor.transpose(
                            pt[:], lnt[:, 128 * f : 128 * (f + 1)], ident[:]
                        )
                        nc.vector.tensor_copy(
                            ln2T[:, f, 128 * u : 128 * (u + 1)], pt[:]
                        )

                # FC + gelu, feature-major
                for g in range(8):
                    wt = wstream.tile([128, 8, TS], F32R, tag="wst")
                    nc.sync.dma_start(
                        wt[:],
                        wfc.rearrange("(h p) f -> p h f", p=128)[
                            :, :, TS * g : TS * (g + 1)
                        ],
                    )
                    for c4 in range(4):
                        fc = 4 * g + c4
                        pf = psC.tile([128, TS], F32, tag="pf")
                        for ht in range(8):
                            nc.tensor.matmul(
                                pf[:],
                                wt[:, ht, 128 * c4 : 128 * (c4 + 1)],
                                ln2T[:, ht, :],
                                start=(ht == 0),
                                stop=(ht == 7),
                            )
                        nc.scalar.activation(
                            h1T[:, fc, :], pf[:], AF.Gelu, bias=bfc_sb[:, fc : fc + 1]
                        )

            # proj back to H, token-major, 8 accumulators across streamed weights
            with tc.tile_pool(name="psP", bufs=8, space="PSUM") as psP:
                pps = [psP.tile([128, TS], F32, tag="pp", name=f"pp{i}")
                       for i in range(8)]
                for g in range(8):
                    wt = wstream.tile([128, 4, H], F32R, tag="wst")
                    nc.sync.dma_start(
                        wt[:],
                        wproj.rearrange("(c p) f -> p c f", p=128)[
                            :, 4 * g : 4 * (g + 1), :
                        ],
                    )
                    for u in range(4):
                        for oc in range(2):
                            for f4 in range(4):
                                fc = 4 * g + f4
                                nc.tensor.matmul(
                                    pps[2 * u + oc][:],
                                    h1T[:, fc, 128 * u : 128 * (u + 1)],
                                    wt[:, f4, TS * oc : TS * (oc + 1)],
                                    start=(g == 0 and f4 == 0),
                                    stop=(g == 7 and f4 == 3),
                                    skip_group_check=True,
                                )
                for u in range(4):
                    for oc in range(2):
                        osl = slice(TS * oc, TS * (oc + 1))
                        t1 = scratch2.tile([128, H], F32, tag="t1")
                        nc.vector.tensor_add(
                            t1[:, 0:TS], pps[2 * u + oc][:], x2[:, u, osl]
                        )
                        ro = outp.tile([128, TS], F32, tag="ro")
                        nc.vector.tensor_add(ro[:], t1[:, 0:TS], bproj_b[:, osl])
                        nc.sync.dma_start(out[128 * u : 128 * (u + 1), osl], ro[:])


_CACHE = {}


def _get_compiled():
    if "nc" not in _CACHE:
        nc = bacc.Bacc("TRN2", target_bir_lowering=False, debug=False,
                       num_devices=NCORES)
        build(nc)
        nc.compile()
        _CACHE["nc"] = nc
    return _CACHE["nc"]


def _make_cmask():
    i = np.arange(128)[:, None]
    j = np.arange(TS)[None, :]
    m = np.zeros((128, 4, TS), np.float32)
    for dd in range(4):
        m[:, dd, :] = (128 * dd + i <= j).astype(np.float32)
    return np.ascontiguousarray(m.reshape(128, 4 * TS))


def make_in_maps(x, ln1_w, ln1_b, W_qkv, b_qkv, W_o, b_o, ln2_w, ln2_b, W_fc,
                 b_fc, W_proj, b_proj):
    x = np.ascontiguousarray(np.asarray(x, np.float32))
    W_qkv = np.asarray(W_qkv, np.float32)
    b_qkv = np.asarray(b_qkv, np.float32)
    cm = _make_cmask()
    ln1wb = np.stack([np.asarray(ln1_w, np.float32), np.asarray(ln1_b, np.float32)])
    ln2wb = np.stack([np.asarray(ln2_w, np.float32), np.asarray(ln2_b, np.float32)])
    shared = {
        "ln1wb": ln1wb,
        "ln2wb": ln2wb,
        "cmask": cm,
        "bo": np.asarray(b_o, np.float32).reshape(1, H),
        "wfc": np.ascontiguousarray(np.asarray(W_fc, np.float32)),
        "bfc": np.ascontiguousarray(np.asarray(b_fc, np.float32).reshape(FF // 128, 128)),
        "wproj": np.ascontiguousarray(np.asarray(W_proj, np.float32)),
        "bproj": np.asarray(b_proj, np.float32).reshape(1, H),
    }
    in_maps = []
    for c in range(NCORES):
        b, r = c // TP, c % TP
        fsl = slice(FQ * r, FQ * (r + 1))
        m = dict(shared)
        m["x"] = x[b]
        m["xs"] = np.ascontiguousarray(x[b][TS * r : TS * (r + 1)])
        m["wq"] = np.ascontiguousarray(W_qkv[:, fsl])
        m["wk"] = np.ascontiguousarray(W_qkv[:, H:][:, fsl])
        m["wv"] = np.ascontiguousarray(W_qkv[:, 2 * H :][:, fsl])
        m["bq"] = np.ascontiguousarray(b_qkv[fsl].reshape(2, 128))
        m["bk"] = np.ascontiguousarray(b_qkv[H:][fsl].reshape(2, 128))
        m["bv"] = np.ascontiguousarray(b_qkv[2 * H :][fsl].reshape(1, FQ))
        m["wo"] = np.ascontiguousarray(np.asarray(W_o, np.float32)[fsl, :])
        in_maps.append(m)
    return in_maps


def kernel(**inputs):
    nc = _get_compiled()
    in_maps = make_in_maps(**inputs)
    res = bass_utils.run_bass_kernel_spmd(
        nc, in_maps, core_ids=list(range(NCORES)), trace=False
    )
    out = np.empty((B, S, H), np.float32)
    for c in range(NCORES):
        b, r = c // TP, c % TP
        out[b, TS * r : TS * (r + 1), :] = res.results[c]["out"]
    return out
